# revision 5
# baseline (speedup 1.0000x reference)
"""Trainium2 Bass kernel for a channel-attention encoder block.

Problem (hardcoded): C=8,S=512,D=512,H=8,FF=2048, N=4096 tokens packed into
4 independent segments of 1024 tokens (seg_ids = arange // 1024), pre-LN MHA
with 2D axial RoPE + block-diagonal softmax + pre-LN FFN (tanh-gelu).

Sharding: 8 cores = 4 segments x 2 query-halves. Each core computes the full
segment's K/V (QKV redundantly over 1024 tokens) and attends its own 512
queries; FFN on its own 512 rows. No collectives. Per-core token order is
host-permuted so the core's own 512 query rows come first, making the device
program identical on all cores (softmax is permutation-invariant over keys).

All big matmuls run in float32r (fp32 rounded to 11 mantissa bits, full PE
rate). LayerNorm affine params are folded into the following weight matrix
host-side. RoPE cos/sin tables are precomputed host-side. Softmax skips the
max-subtraction (scores are O(1) here; exp is safe in fp32) so the key-axis
reduction folds into the attention matmuls via an appended ones column on v.
"""

import sys

for _p in ("/opt/trn_rl_repo", "/root/.axon_site/_ro/trn_rl_repo"):
    if _p not in sys.path:
        sys.path.append(_p)

import numpy as np

import concourse.bass as bass
import concourse.mybir as mybir
import concourse.tile as tile
from concourse import bacc
from concourse.bass_utils import run_bass_kernel_spmd
from concourse.masks import make_identity

P = 128
D = 512
H = 8
DH = 64
FF = 2048
SEG = 1024
QT = 512           # own query tokens per core
NCORES = 8
N = 4096
F32 = mybir.dt.float32
F32R = mybir.dt.float32r
AF = mybir.ActivationFunctionType
OP = mybir.AluOpType

KC_D = D // P      # 4   contraction chunks over D
TT_KV = SEG // P   # 8   kv token tiles
TT_Q = QT // P     # 4   own query token tiles
FFC = FF // P      # 16  FF feature chunks


def _round_fp32r(x: np.ndarray) -> np.ndarray:
    """Round fp32 to 11 mantissa bits (hardware float32r), RNE."""
    b = np.ascontiguousarray(x, dtype=np.float32).view(np.uint32).astype(np.uint64)
    lsb = (b >> 12) & 1
    b = (b + 0x7FF + lsb) & 0xFFFFF000
    return b.astype(np.uint32).view(np.float32)


def _build_nc():
    nc = bacc.Bacc("TRN2", target_bir_lowering=False)

    x_kv = nc.dram_tensor("x_kv", [SEG, D], F32, kind="ExternalInput")
    ck = nc.dram_tensor("ck", [SEG, DH], F32, kind="ExternalInput")
    sk = nc.dram_tensor("sk", [SEG, DH], F32, kind="ExternalInput")
    wqkv = nc.dram_tensor("wqkv", [P, KC_D, 3 * D], F32R, kind="ExternalInput")
    bqkv = nc.dram_tensor("bqkv", [3 * D], F32, kind="ExternalInput")
    wo = nc.dram_tensor("wo", [P, KC_D, D], F32R, kind="ExternalInput")
    bo = nc.dram_tensor("bo", [D], F32, kind="ExternalInput")
    w1 = nc.dram_tensor("w1", [P, KC_D, FF], F32R, kind="ExternalInput")
    b1 = nc.dram_tensor("b1", [FF], F32, kind="ExternalInput")
    w2 = nc.dram_tensor("w2", [P, FFC, D], F32R, kind="ExternalInput")
    b2 = nc.dram_tensor("b2", [D], F32, kind="ExternalInput")
    y = nc.dram_tensor("y", [QT, D], F32, kind="ExternalOutput")

    with tile.TileContext(nc) as tc:
        _emit(nc, tc, x_kv, ck, sk, wqkv, bqkv, wo, bo, w1, b1, w2, b2, y)
    nc.compile()
    return nc


def _emit(nc, tc, x_kv, ck, sk, wqkv, bqkv, wo, bo, w1, b1, w2, b2, y):
    from contextlib import ExitStack

    top = ExitStack()
    with top:
        const = top.enter_context(tc.tile_pool(name="const", bufs=1))
        persist = top.enter_context(tc.tile_pool(name="persist", bufs=1))

        # ---- constants ----
        ident = const.tile([P, P], F32)
        make_identity(nc, ident)
        ones_row = const.tile([P, DH], F32R)
        nc.vector.memset(ones_row[:].bitcast(F32), 1.0)
        eps = const.tile([P, 1], F32)
        nc.vector.memset(eps[:], 1e-5)
        bqkv_bc = const.tile([P, 3 * D], F32)
        nc.sync.dma_start(bqkv_bc[:], bqkv.ap()[None, :].to_broadcast((P, 3 * D)))
        bo_bc = const.tile([P, D], F32)
        nc.sync.dma_start(bo_bc[:], bo.ap()[None, :].to_broadcast((P, D)))
        b2_bc = const.tile([P, D], F32)
        nc.sync.dma_start(b2_bc[:], b2.ap()[None, :].to_broadcast((P, D)))
        b1_pc = const.tile([P, FFC], F32)
        nc.sync.dma_start(b1_pc[:], b1.ap().rearrange("(c p) -> p c", p=P))
        ck_sb = const.tile([P, TT_KV, DH], F32)
        nc.sync.dma_start(ck_sb[:], ck.ap().rearrange("(t p) d -> p t d", p=P))
        sk_sb = const.tile([P, TT_KV, DH], F32)
        nc.sync.dma_start(sk_sb[:], sk.ap().rearrange("(t p) d -> p t d", p=P))

        # ---- persistent activations ----
        qT = persist.tile([P, KC_D, QT], F32R)           # q feature-major
        kT = persist.tile([P, KC_D, SEG], F32R)          # k feature-major
        vaug = persist.tile([P, TT_KV, H, DH + 1], F32R)  # [v | 1] per kv tile
        oT = persist.tile([P, KC_D, QT], F32R)           # attn out feature-major
        h1 = persist.tile([P, TT_Q, D], F32)             # post-attn residual
        hn2T = persist.tile([P, KC_D, QT], F32R)         # LN2 out transposed

        # =========== Phase A: LN1 + QKV + RoPE + transposes ===========
        with ExitStack() as sa:
            pa = sa.enter_context(tc.tile_pool(name="pa", bufs=3))
            wpool = sa.enter_context(tc.tile_pool(name="wpool", bufs=1))
            hnTp = sa.enter_context(tc.tile_pool(name="hnTp", bufs=1))
            psA = sa.enter_context(tc.tile_pool(name="psA", bufs=2, space="PSUM"))
            psQ = sa.enter_context(tc.tile_pool(name="psQ", bufs=3, space="PSUM"))

            wqkv_sb = wpool.tile([P, KC_D, 3 * D], F32R)
            nc.sync.dma_start(wqkv_sb[:], wqkv.ap())
            hnT = hnTp.tile([P, KC_D, SEG], F32R)

            for tt in range(TT_KV):
                xt = pa.tile([P, D], F32, name="xt", tag="xt")
                nc.sync.dma_start(xt[:], x_kv.ap()[tt * P:(tt + 1) * P, :])
                stats = pa.tile([P, 6], F32, name="stats", tag="stats")
                nc.vector.bn_stats(stats[:], xt[:])
                mv = pa.tile([P, 2], F32, name="mv", tag="mv")
                nc.vector.bn_aggr(mv[:], stats[:])
                rstd = pa.tile([P, 1], F32, name="rstd", tag="rstd")
                nc.scalar.activation(rstd[:], mv[:, 1:2], AF.Sqrt, bias=eps[:])
                nc.vector.reciprocal(rstd[:], rstd[:])
                hn = pa.tile([P, D], F32, name="hn", tag="hn")
                nc.vector.tensor_scalar(
                    hn[:], xt[:], mv[:, 0:1], rstd[:], OP.subtract, OP.mult)
                for fc in range(KC_D):
                    pt = psA.tile([P, P], F32, name="pt", tag="pt")
                    nc.tensor.transpose(pt[:], hn[:, fc * P:(fc + 1) * P], ident[:])
                    nc.any.tensor_copy(hnT[:, fc, tt * P:(tt + 1) * P], pt[:])

                # QKV for this token tile (q only for own tiles 0..3)
                fjs = ([0] if tt < TT_Q else []) + [1, 2]
                for fj in fjs:
                    pq = psQ.tile([P, D], F32, name="pq", tag="pq")
                    for kc in range(KC_D):
                        nc.tensor.matmul(
                            pq[:], hnT[:, kc, tt * P:(tt + 1) * P],
                            wqkv_sb[:, kc, fj * D:(fj + 1) * D],
                            start=(kc == 0), stop=(kc == KC_D - 1))
                    if fj == 2:
                        # v -> vaug: per head [v (64) | 1]
                        vg = vaug[:, tt]
                        nc.vector.memset(vg[:, :, DH:DH + 1].bitcast(F32), 1.0)
                        pqh = pq[:].rearrange("p (h d) -> p h d", h=H)
                        bvh = bqkv_bc[:, 2 * D:3 * D].rearrange(
                            "p (h d) -> p h d", h=H)
                        nc.any.tensor_tensor(
                            vg[:, :, 0:DH], pqh[:], bvh[:], OP.add)
                    else:
                        # q/k: bias add -> RoPE -> transpose to qT/kT
                        qk = pa.tile([P, D], F32, name="qk", tag="qk")
                        nc.any.tensor_tensor(
                            qk[:], pq[:],
                            bqkv_bc[:, fj * D:(fj + 1) * D], OP.add)
                        qkv4 = qk[:].rearrange("p (h g s) -> p h g s", h=H, s=16)
                        tmp = pa.tile([P, D], F32, name="tmp", tag="tmp")
                        tmp4 = tmp[:].rearrange("p (h g s) -> p h g s", h=H, s=16)
                        nc.any.tensor_copy(tmp4[:, :, 0::2, :], qkv4[:, :, 1::2, :])
                        nc.any.tensor_copy(tmp4[:, :, 1::2, :], qkv4[:, :, 0::2, :])
                        rot = pa.tile([P, D], F32, name="rot", tag="rot")
                        roth = rot[:].rearrange("p (h d) -> p h d", h=H)
                        qkh = qk[:].rearrange("p (h d) -> p h d", h=H)
                        tmph = tmp[:].rearrange("p (h d) -> p h d", h=H)
                        cbc = ck_sb[:, tt:tt + 1, :].to_broadcast((P, H, DH))
                        sbc = sk_sb[:, tt:tt + 1, :].to_broadcast((P, H, DH))
                        nc.any.tensor_tensor(roth, qkh, cbc, OP.mult)
                        nc.any.tensor_tensor(tmph, tmph, sbc, OP.mult)
                        nc.any.tensor_tensor(roth, roth, tmph, OP.add)
                        dstT = qT if fj == 0 else kT
                        for fc in range(KC_D):
                            pt2 = psA.tile([P, P], F32, name="pt2", tag="pt")
                            nc.tensor.transpose(
                                pt2[:], rot[:, fc * P:(fc + 1) * P], ident[:])
                            nc.any.tensor_copy(
                                dstT[:, fc, tt * P:(tt + 1) * P], pt2[:])

        # =========== Phase B: attention ===========
        with ExitStack() as sb_:
            pb = sb_.enter_context(tc.tile_pool(name="pb", bufs=2))
            psS = sb_.enter_context(tc.tile_pool(name="psS", bufs=3, space="PSUM"))
            psO = sb_.enter_context(tc.tile_pool(name="psO", bufs=3, space="PSUM"))
            psX = sb_.enter_context(tc.tile_pool(name="psX", bufs=2, space="PSUM"))

            for fc in range(KC_D):
                recip = pb.tile([P, 2, QT], F32R, name="recip", tag="recip")
                pso_pair = []
                for par in (0, 1):
                    h = 2 * fc + par
                    base = par * DH
                    eT = pb.tile([P, TT_KV, QT], F32R, name="eT", tag="eT")
                    for kc in range(TT_KV):
                        ps_s = psS.tile([P, QT], F32, name="ps_s", tag="ps_s")
                        nc.tensor.matmul(
                            ps_s[:],
                            kT[base:base + DH, fc, kc * P:(kc + 1) * P],
                            qT[base:base + DH, fc, :],
                            start=True, stop=True)
                        nc.scalar.activation(
                            eT[:, kc, :], ps_s[:], AF.Exp, scale=0.125)
                    ps_o = psO.tile([P, QT], F32, name="ps_o", tag="ps_o")
                    pso_pair.append(ps_o)
                    for kc in range(TT_KV):
                        nc.tensor.matmul(
                            ps_o[0:DH + 1, :], vaug[:, kc, h, :], eT[:, kc, :],
                            start=(kc == 0), stop=(kc == TT_KV - 1))
                    with nc.allow_low_precision(reason="fp32r denom recip"):
                        nc.vector.reciprocal(
                            recip[DH:DH + 1, par, :], ps_o[DH:DH + 1, :])
                for par in (0, 1):
                    ps_sc = psX.tile([P, QT], F32, name="ps_sc", tag="ps_sc")
                    nc.tensor.matmul(
                        ps_sc[0:DH, :], ones_row[DH:DH + 1, :],
                        recip[DH:DH + 1, par, :], start=True, stop=True)
                    sc_sb = pb.tile([P, QT], F32, name="sc_sb", tag="sc_sb")
                    nc.any.tensor_copy(sc_sb[0:DH, :], ps_sc[0:DH, :])
                    if par == 0:
                        nc.any.tensor_tensor(
                            oT[0:DH, fc, :], pso_pair[0][0:DH, :],
                            sc_sb[0:DH, :], OP.mult)
                    else:
                        odd_sb = pb.tile([P, QT], F32R, name="odd_sb",
                                         tag="odd_sb")
                        nc.any.tensor_tensor(
                            odd_sb[0:DH, :], pso_pair[1][0:DH, :],
                            sc_sb[0:DH, :], OP.mult)
                        nc.sync.dma_start(oT[DH:P, fc, :], odd_sb[0:DH, :])

        # =========== Phase C: Wo + residual + LN2 + transpose ===========
        with ExitStack() as sc:
            pc = sc.enter_context(tc.tile_pool(name="pc", bufs=3))
            wop = sc.enter_context(tc.tile_pool(name="wop", bufs=1))
            psC = sc.enter_context(tc.tile_pool(name="psC", bufs=2, space="PSUM"))
            psT2 = sc.enter_context(tc.tile_pool(name="psT2", bufs=2, space="PSUM"))

            wo_sb = wop.tile([P, KC_D, D], F32R)
            nc.sync.dma_start(wo_sb[:], wo.ap())
            for tq in range(TT_Q):
                ps_w = psC.tile([P, D], F32, name="ps_w", tag="ps_w")
                for kc in range(KC_D):
                    nc.tensor.matmul(
                        ps_w[:], oT[:, kc, tq * P:(tq + 1) * P], wo_sb[:, kc, :],
                        start=(kc == 0), stop=(kc == KC_D - 1))
                xt2 = pc.tile([P, D], F32, name="xt2", tag="xt2")
                nc.sync.dma_start(xt2[:], x_kv.ap()[tq * P:(tq + 1) * P, :])
                nc.any.tensor_tensor(h1[:, tq, :], ps_w[:], bo_bc[:], OP.add)
                nc.any.tensor_tensor(h1[:, tq, :], h1[:, tq, :], xt2[:], OP.add)
                stats2 = pc.tile([P, 6], F32, name="stats2", tag="stats2")
                nc.vector.bn_stats(stats2[:], h1[:, tq, :])
                mv2 = pc.tile([P, 2], F32, name="mv2", tag="mv2")
                nc.vector.bn_aggr(mv2[:], stats2[:])
                rstd2 = pc.tile([P, 1], F32, name="rstd2", tag="rstd2")
                nc.scalar.activation(rstd2[:], mv2[:, 1:2], AF.Sqrt, bias=eps[:])
                nc.vector.reciprocal(rstd2[:], rstd2[:])
                hn2 = pc.tile([P, D], F32, name="hn2", tag="hn2")
                nc.vector.tensor_scalar(
                    hn2[:], h1[:, tq, :], mv2[:, 0:1], rstd2[:],
                    OP.subtract, OP.mult)
                for fc in range(KC_D):
                    pt3 = psT2.tile([P, P], F32, name="pt3", tag="pt3")
                    nc.tensor.transpose(pt3[:], hn2[:, fc * P:(fc + 1) * P], ident[:])
                    nc.any.tensor_copy(hn2T[:, fc, tq * P:(tq + 1) * P], pt3[:])

        # =========== Phase D: FFN ===========
        with ExitStack() as sd:
            pd = sd.enter_context(tc.tile_pool(name="pd", bufs=3))
            psG = sd.enter_context(tc.tile_pool(name="psG", bufs=2, space="PSUM"))
            psF = sd.enter_context(tc.tile_pool(name="psF", bufs=4, space="PSUM"))

            ps_f = [psF.tile([P, D], F32, name=f"ps_f{tq}", tag="ps_f")
                    for tq in range(TT_Q)]
            for ffc in range(FFC):
                w1t = pd.tile([P, KC_D, P], F32R, name="w1t", tag="w1t")
                nc.sync.dma_start(w1t[:], w1.ap()[:, :, ffc * P:(ffc + 1) * P])
                ps_g = psG.tile([P, QT], F32, name="ps_g", tag="ps_g")
                for kc in range(KC_D):
                    nc.tensor.matmul(
                        ps_g[:], w1t[:, kc, :], hn2T[:, kc, :],
                        start=(kc == 0), stop=(kc == KC_D - 1))
                gt = pd.tile([P, QT], F32R, name="gt", tag="gt")
                nc.scalar.activation(
                    gt[:], ps_g[:], AF.Gelu_apprx_tanh,
                    bias=b1_pc[:, ffc:ffc + 1])
                w2t = pd.tile([P, D], F32R, name="w2t", tag="w2t")
                nc.sync.dma_start(w2t[:], w2.ap()[:, ffc, :])
                for tq in range(TT_Q):
                    nc.tensor.matmul(
                        ps_f[tq][:], gt[:, tq * P:(tq + 1) * P], w2t[:],
                        start=(ffc == 0), stop=(ffc == FFC - 1))
            for tq in range(TT_Q):
                yt = pd.tile([P, D], F32, name="yt", tag="yt")
                nc.any.tensor_tensor(yt[:], ps_f[tq][:], b2_bc[:], OP.add)
                nc.any.tensor_tensor(yt[:], yt[:], h1[:, tq, :], OP.add)
                nc.sync.dma_start(y.ap()[tq * P:(tq + 1) * P, :], yt[:])


_NC_CACHE = {}


def _get_nc():
    if "nc" not in _NC_CACHE:
        _NC_CACHE["nc"] = _build_nc()
    return _NC_CACHE["nc"]


def _host_prep(x, pos, Wqkv, bqkv, Wo, bo, ln1_w, ln1_b, ln2_w, ln2_b,
               W1, b1, W2, b2):
    """Fold LN affines into weights, build RoPE tables, shape weights."""
    xf = np.asarray(x, np.float32).reshape(N, D)
    pf = np.asarray(pos, np.float32).reshape(N, 2)

    Wqkv = np.asarray(Wqkv, np.float32)
    W1 = np.asarray(W1, np.float32)
    wqkv_f = np.asarray(ln1_w, np.float64)[:, None] * Wqkv.astype(np.float64)
    bqkv_f = np.asarray(ln1_b, np.float64) @ Wqkv.astype(np.float64) + bqkv
    w1_f = np.asarray(ln2_w, np.float64)[:, None] * W1.astype(np.float64)
    b1_f = np.asarray(ln2_b, np.float64) @ W1.astype(np.float64) + b1

    def shape_w(w, kin):
        w = np.asarray(w, np.float32)
        ko = kin // P
        return _round_fp32r(
            np.ascontiguousarray(w.reshape(ko, P, -1).transpose(1, 0, 2)))

    wqkv_r = shape_w(wqkv_f, D)
    wo_r = shape_w(Wo, D)
    w1_r = shape_w(w1_f, D)
    w2_r = shape_w(W2, FF)

    # RoPE tables (sign-folded):  out = t*C + shuf16(t)*S
    hd2 = DH // 2
    inv_freq = (1.0 / (10000.0 ** (np.arange(0, hd2, 2, dtype=np.float64) / hd2)))
    f0 = pf[:, 0:1].astype(np.float64) * inv_freq[None, :]   # (N, 16)
    f1 = pf[:, 1:2].astype(np.float64) * inv_freq[None, :]
    C = np.concatenate(
        [np.cos(f0), np.cos(f0), np.cos(f1), np.cos(f1)], axis=1)
    S = np.concatenate(
        [-np.sin(f0), np.sin(f0), -np.sin(f1), np.sin(f1)], axis=1)
    return (xf, C.astype(np.float32), S.astype(np.float32),
            wqkv_r, np.asarray(bqkv_f, np.float32),
            wo_r, np.asarray(bo, np.float32),
            w1_r, np.asarray(b1_f, np.float32),
            w2_r, np.asarray(b2, np.float32))


def _in_maps(inputs):
    (xf, C, S, wqkv_r, bqkv_f, wo_r, bo_f, w1_r, b1_f, w2_r, b2_f) = _host_prep(
        inputs["x"], inputs["pos"], inputs["Wqkv"], inputs["bqkv"],
        inputs["Wo"], inputs["bo"], inputs["ln1_w"], inputs["ln1_b"],
        inputs["ln2_w"], inputs["ln2_b"], inputs["W1"], inputs["b1"],
        inputs["W2"], inputs["b2"])
    shared = dict(wqkv=wqkv_r, bqkv=bqkv_f, wo=wo_r, bo=bo_f,
                  w1=w1_r, b1=b1_f, w2=w2_r, b2=b2_f)
    maps = []
    for c in range(NCORES):
        seg, half = c // 2, c % 2
        own = np.arange(seg * SEG + half * QT, seg * SEG + half * QT + QT)
        other = np.arange(seg * SEG + (1 - half) * QT,
                          seg * SEG + (1 - half) * QT + QT)
        perm = np.concatenate([own, other])
        maps.append(dict(shared,
                         x_kv=np.ascontiguousarray(xf[perm]),
                         ck=np.ascontiguousarray(C[perm]),
                         sk=np.ascontiguousarray(S[perm])))
    return maps


def kernel(**inputs) -> np.ndarray:
    nc = _get_nc()
    maps = _in_maps(inputs)
    res = run_bass_kernel_spmd(nc, maps, list(range(NCORES)))
    out = np.empty((N, D), np.float32)
    for c in range(NCORES):
        seg, half = c // 2, c % 2
        q0 = seg * SEG + half * QT
        out[q0:q0 + QT] = res.results[c]["y"]
    return out.reshape(8, 512, D)


# revision 6
# speedup vs baseline: 384.0352x; 384.0352x over previous
"""Trainium2 Bass kernel for a channel-attention encoder block.

Problem (hardcoded): C=8,S=512,D=512,H=8,FF=2048, N=4096 tokens packed into
4 independent segments of 1024 tokens (seg_ids = arange // 1024), pre-LN MHA
with 2D axial RoPE + block-diagonal softmax + pre-LN FFN (tanh-gelu).

Sharding: 8 cores = 4 segments x 2 query-halves. Each core computes the full
segment's K/V (QKV redundantly over 1024 tokens) and attends its own 512
queries; FFN on its own 512 rows. No collectives. Per-core token order is
host-permuted so the core's own 512 query rows come first, making the device
program identical on all cores (softmax is permutation-invariant over keys).

All big matmuls run in float32r (fp32 rounded to 11 mantissa bits, full PE
rate). LayerNorm affine params are folded into the following weight matrix
host-side. RoPE cos/sin tables are precomputed host-side. Softmax skips the
max-subtraction (scores are O(1) here; exp is safe in fp32) so the key-axis
reduction folds into the attention matmuls via an appended ones column on v.
"""

import sys

for _p in ("/opt/trn_rl_repo", "/root/.axon_site/_ro/trn_rl_repo"):
    if _p not in sys.path:
        sys.path.append(_p)

import numpy as np

import concourse.bass as bass
import concourse.mybir as mybir
import concourse.tile as tile
from concourse import bacc
from concourse.bass_utils import run_bass_kernel_spmd
from concourse.masks import make_identity

P = 128
D = 512
H = 8
DH = 64
FF = 2048
SEG = 1024
QT = 512           # own query tokens per core
NCORES = 8
N = 4096
F32 = mybir.dt.float32
F32R = mybir.dt.float32r
AF = mybir.ActivationFunctionType
OP = mybir.AluOpType

KC_D = D // P      # 4   contraction chunks over D
TT_KV = SEG // P   # 8   kv token tiles
TT_Q = QT // P     # 4   own query token tiles
FFC = FF // P      # 16  FF feature chunks


def _round_fp32r(x: np.ndarray) -> np.ndarray:
    """Round fp32 to 11 mantissa bits (hardware float32r), RNE."""
    b = np.ascontiguousarray(x, dtype=np.float32).view(np.uint32).astype(np.uint64)
    lsb = (b >> 12) & 1
    b = (b + 0x7FF + lsb) & 0xFFFFF000
    return b.astype(np.uint32).view(np.float32)


def _build_nc(loop_n=None):
    nc = bacc.Bacc("TRN2", target_bir_lowering=False)

    x_kv = nc.dram_tensor("x_kv", [SEG, D], F32, kind="ExternalInput")
    ck = nc.dram_tensor("ck", [SEG, DH], F32, kind="ExternalInput")
    sk = nc.dram_tensor("sk", [SEG, DH], F32, kind="ExternalInput")
    wqkv = nc.dram_tensor("wqkv", [P, KC_D, 3 * D], F32R, kind="ExternalInput")
    bqkv = nc.dram_tensor("bqkv", [3 * D], F32, kind="ExternalInput")
    wo = nc.dram_tensor("wo", [P, KC_D, D], F32R, kind="ExternalInput")
    bo = nc.dram_tensor("bo", [D], F32, kind="ExternalInput")
    w1 = nc.dram_tensor("w1", [P, KC_D, FF], F32R, kind="ExternalInput")
    b1 = nc.dram_tensor("b1", [FF], F32, kind="ExternalInput")
    w2 = nc.dram_tensor("w2", [P, FFC, D], F32R, kind="ExternalInput")
    b2 = nc.dram_tensor("b2", [D], F32, kind="ExternalInput")
    y = nc.dram_tensor("y", [QT, D], F32, kind="ExternalOutput")

    with tile.TileContext(nc) as tc:
        if loop_n is None:
            _emit(nc, tc, x_kv, ck, sk, wqkv, bqkv, wo, bo, w1, b1, w2, b2, y)
        else:
            with tc.For_i(0, loop_n, 1):
                _emit(nc, tc, x_kv, ck, sk, wqkv, bqkv, wo, bo, w1, b1, w2,
                      b2, y)
    nc.compile()
    return nc


def _emit(nc, tc, x_kv, ck, sk, wqkv, bqkv, wo, bo, w1, b1, w2, b2, y):
    from contextlib import ExitStack

    top = ExitStack()
    with top:
        const = top.enter_context(tc.tile_pool(name="const", bufs=1))
        persist = top.enter_context(tc.tile_pool(name="persist", bufs=1))

        # ---- constants ----
        ident = const.tile([P, P], F32)
        make_identity(nc, ident)
        ones_row = const.tile([P, DH], F32R)
        nc.vector.memset(ones_row[:].bitcast(F32), 1.0)
        eps = const.tile([P, 1], F32)
        nc.vector.memset(eps[:], 1e-5)
        bqkv_bc = const.tile([P, 3 * D], F32)
        nc.sync.dma_start(bqkv_bc[:], bqkv.ap()[None, :].to_broadcast((P, 3 * D)))
        bo_bc = const.tile([P, D], F32)
        nc.sync.dma_start(bo_bc[:], bo.ap()[None, :].to_broadcast((P, D)))
        b2_bc = const.tile([P, D], F32)
        nc.sync.dma_start(b2_bc[:], b2.ap()[None, :].to_broadcast((P, D)))
        b1_pc = const.tile([P, FFC], F32)
        nc.sync.dma_start(b1_pc[:], b1.ap().rearrange("(c p) -> p c", p=P))
        ck_sb = const.tile([P, TT_KV, DH], F32)
        nc.sync.dma_start(ck_sb[:], ck.ap().rearrange("(t p) d -> p t d", p=P))
        sk_sb = const.tile([P, TT_KV, DH], F32)
        nc.sync.dma_start(sk_sb[:], sk.ap().rearrange("(t p) d -> p t d", p=P))

        # ---- persistent activations ----
        qT = persist.tile([P, KC_D, QT], F32R)           # q feature-major
        kT = persist.tile([P, KC_D, SEG], F32R)          # k feature-major
        vaug = persist.tile([P, TT_KV, H, DH + 1], F32R)  # [v | 1] per kv tile
        oT = persist.tile([P, KC_D, QT], F32R)           # attn out feature-major
        h1 = persist.tile([P, TT_Q, D], F32)             # post-attn residual
        hn2T = persist.tile([P, KC_D, QT], F32R)         # LN2 out transposed

        # =========== Phase A: LN1 + QKV + RoPE + transposes ===========
        with ExitStack() as sa:
            pa = sa.enter_context(tc.tile_pool(name="pa", bufs=3))
            wpool = sa.enter_context(tc.tile_pool(name="wpool", bufs=1))
            hnTp = sa.enter_context(tc.tile_pool(name="hnTp", bufs=1))
            psA = sa.enter_context(tc.tile_pool(name="psA", bufs=2, space="PSUM"))
            psQ = sa.enter_context(tc.tile_pool(name="psQ", bufs=3, space="PSUM"))

            wqkv_sb = wpool.tile([P, KC_D, 3 * D], F32R)
            nc.sync.dma_start(wqkv_sb[:], wqkv.ap())
            hnT = hnTp.tile([P, KC_D, SEG], F32R)

            for tt in range(TT_KV):
                xt = pa.tile([P, D], F32, name="xt", tag="xt")
                nc.sync.dma_start(xt[:], x_kv.ap()[tt * P:(tt + 1) * P, :])
                stats = pa.tile([P, 6], F32, name="stats", tag="stats")
                nc.vector.bn_stats(stats[:], xt[:])
                mv = pa.tile([P, 2], F32, name="mv", tag="mv")
                nc.vector.bn_aggr(mv[:], stats[:])
                rstd = pa.tile([P, 1], F32, name="rstd", tag="rstd")
                nc.scalar.activation(rstd[:], mv[:, 1:2], AF.Sqrt, bias=eps[:])
                nc.vector.reciprocal(rstd[:], rstd[:])
                hn = pa.tile([P, D], F32, name="hn", tag="hn")
                nc.vector.tensor_scalar(
                    hn[:], xt[:], mv[:, 0:1], rstd[:], OP.subtract, OP.mult)
                for fc in range(KC_D):
                    pt = psA.tile([P, P], F32, name="pt", tag="pt")
                    nc.tensor.transpose(pt[:], hn[:, fc * P:(fc + 1) * P], ident[:])
                    nc.any.tensor_copy(hnT[:, fc, tt * P:(tt + 1) * P], pt[:])

                # QKV for this token tile (q only for own tiles 0..3)
                fjs = ([0] if tt < TT_Q else []) + [1, 2]
                for fj in fjs:
                    pq = psQ.tile([P, D], F32, name="pq", tag="pq")
                    for kc in range(KC_D):
                        nc.tensor.matmul(
                            pq[:], hnT[:, kc, tt * P:(tt + 1) * P],
                            wqkv_sb[:, kc, fj * D:(fj + 1) * D],
                            start=(kc == 0), stop=(kc == KC_D - 1))
                    if fj == 2:
                        # v -> vaug: per head [v (64) | 1]
                        vg = vaug[:, tt]
                        nc.vector.memset(vg[:, :, DH:DH + 1].bitcast(F32), 1.0)
                        pqh = pq[:].rearrange("p (h d) -> p h d", h=H)
                        bvh = bqkv_bc[:, 2 * D:3 * D].rearrange(
                            "p (h d) -> p h d", h=H)
                        nc.any.tensor_tensor(
                            vg[:, :, 0:DH], pqh[:], bvh[:], OP.add)
                    else:
                        # q/k: bias add -> RoPE -> transpose to qT/kT
                        qk = pa.tile([P, D], F32, name="qk", tag="qk")
                        nc.any.tensor_tensor(
                            qk[:], pq[:],
                            bqkv_bc[:, fj * D:(fj + 1) * D], OP.add)
                        qkv4 = qk[:].rearrange("p (h g s) -> p h g s", h=H, s=16)
                        tmp = pa.tile([P, D], F32, name="tmp", tag="tmp")
                        tmp4 = tmp[:].rearrange("p (h g s) -> p h g s", h=H, s=16)
                        nc.any.tensor_copy(tmp4[:, :, 0::2, :], qkv4[:, :, 1::2, :])
                        nc.any.tensor_copy(tmp4[:, :, 1::2, :], qkv4[:, :, 0::2, :])
                        rot = pa.tile([P, D], F32, name="rot", tag="rot")
                        roth = rot[:].rearrange("p (h d) -> p h d", h=H)
                        qkh = qk[:].rearrange("p (h d) -> p h d", h=H)
                        tmph = tmp[:].rearrange("p (h d) -> p h d", h=H)
                        cbc = ck_sb[:, tt:tt + 1, :].to_broadcast((P, H, DH))
                        sbc = sk_sb[:, tt:tt + 1, :].to_broadcast((P, H, DH))
                        nc.any.tensor_tensor(roth, qkh, cbc, OP.mult)
                        nc.any.tensor_tensor(tmph, tmph, sbc, OP.mult)
                        nc.any.tensor_tensor(roth, roth, tmph, OP.add)
                        dstT = qT if fj == 0 else kT
                        for fc in range(KC_D):
                            pt2 = psA.tile([P, P], F32, name="pt2", tag="pt")
                            nc.tensor.transpose(
                                pt2[:], rot[:, fc * P:(fc + 1) * P], ident[:])
                            nc.any.tensor_copy(
                                dstT[:, fc, tt * P:(tt + 1) * P], pt2[:])

        # =========== Phase B: attention ===========
        with ExitStack() as sb_:
            pb = sb_.enter_context(tc.tile_pool(name="pb", bufs=2))
            psS = sb_.enter_context(tc.tile_pool(name="psS", bufs=3, space="PSUM"))
            psO = sb_.enter_context(tc.tile_pool(name="psO", bufs=3, space="PSUM"))
            psX = sb_.enter_context(tc.tile_pool(name="psX", bufs=2, space="PSUM"))

            for fc in range(KC_D):
                recip = pb.tile([P, 2, QT], F32R, name="recip", tag="recip")
                pso_pair = []
                for par in (0, 1):
                    h = 2 * fc + par
                    base = par * DH
                    eT = pb.tile([P, TT_KV, QT], F32R, name="eT", tag="eT")
                    for kc in range(TT_KV):
                        ps_s = psS.tile([P, QT], F32, name="ps_s", tag="ps_s")
                        nc.tensor.matmul(
                            ps_s[:],
                            kT[base:base + DH, fc, kc * P:(kc + 1) * P],
                            qT[base:base + DH, fc, :],
                            start=True, stop=True)
                        nc.scalar.activation(
                            eT[:, kc, :], ps_s[:], AF.Exp, scale=0.125)
                    ps_o = psO.tile([P, QT], F32, name="ps_o", tag="ps_o")
                    pso_pair.append(ps_o)
                    for kc in range(TT_KV):
                        nc.tensor.matmul(
                            ps_o[0:DH + 1, :], vaug[:, kc, h, :], eT[:, kc, :],
                            start=(kc == 0), stop=(kc == TT_KV - 1))
                    with nc.allow_low_precision(reason="fp32r denom recip"):
                        nc.vector.reciprocal(
                            recip[DH:DH + 1, par, :], ps_o[DH:DH + 1, :])
                for par in (0, 1):
                    ps_sc = psX.tile([P, QT], F32, name="ps_sc", tag="ps_sc")
                    nc.tensor.matmul(
                        ps_sc[0:DH, :], ones_row[DH:DH + 1, :],
                        recip[DH:DH + 1, par, :], start=True, stop=True)
                    sc_sb = pb.tile([P, QT], F32, name="sc_sb", tag="sc_sb")
                    nc.any.tensor_copy(sc_sb[0:DH, :], ps_sc[0:DH, :])
                    if par == 0:
                        nc.any.tensor_tensor(
                            oT[0:DH, fc, :], pso_pair[0][0:DH, :],
                            sc_sb[0:DH, :], OP.mult)
                    else:
                        odd_sb = pb.tile([P, QT], F32R, name="odd_sb",
                                         tag="odd_sb")
                        nc.any.tensor_tensor(
                            odd_sb[0:DH, :], pso_pair[1][0:DH, :],
                            sc_sb[0:DH, :], OP.mult)
                        nc.sync.dma_start(oT[DH:P, fc, :], odd_sb[0:DH, :])

        # =========== Phase C: Wo + residual + LN2 + transpose ===========
        with ExitStack() as sc:
            pc = sc.enter_context(tc.tile_pool(name="pc", bufs=3))
            wop = sc.enter_context(tc.tile_pool(name="wop", bufs=1))
            psC = sc.enter_context(tc.tile_pool(name="psC", bufs=2, space="PSUM"))
            psT2 = sc.enter_context(tc.tile_pool(name="psT2", bufs=2, space="PSUM"))

            wo_sb = wop.tile([P, KC_D, D], F32R)
            nc.sync.dma_start(wo_sb[:], wo.ap())
            for tq in range(TT_Q):
                ps_w = psC.tile([P, D], F32, name="ps_w", tag="ps_w")
                for kc in range(KC_D):
                    nc.tensor.matmul(
                        ps_w[:], oT[:, kc, tq * P:(tq + 1) * P], wo_sb[:, kc, :],
                        start=(kc == 0), stop=(kc == KC_D - 1))
                xt2 = pc.tile([P, D], F32, name="xt2", tag="xt2")
                nc.sync.dma_start(xt2[:], x_kv.ap()[tq * P:(tq + 1) * P, :])
                nc.any.tensor_tensor(h1[:, tq, :], ps_w[:], bo_bc[:], OP.add)
                nc.any.tensor_tensor(h1[:, tq, :], h1[:, tq, :], xt2[:], OP.add)
                stats2 = pc.tile([P, 6], F32, name="stats2", tag="stats2")
                nc.vector.bn_stats(stats2[:], h1[:, tq, :])
                mv2 = pc.tile([P, 2], F32, name="mv2", tag="mv2")
                nc.vector.bn_aggr(mv2[:], stats2[:])
                rstd2 = pc.tile([P, 1], F32, name="rstd2", tag="rstd2")
                nc.scalar.activation(rstd2[:], mv2[:, 1:2], AF.Sqrt, bias=eps[:])
                nc.vector.reciprocal(rstd2[:], rstd2[:])
                hn2 = pc.tile([P, D], F32, name="hn2", tag="hn2")
                nc.vector.tensor_scalar(
                    hn2[:], h1[:, tq, :], mv2[:, 0:1], rstd2[:],
                    OP.subtract, OP.mult)
                for fc in range(KC_D):
                    pt3 = psT2.tile([P, P], F32, name="pt3", tag="pt3")
                    nc.tensor.transpose(pt3[:], hn2[:, fc * P:(fc + 1) * P], ident[:])
                    nc.any.tensor_copy(hn2T[:, fc, tq * P:(tq + 1) * P], pt3[:])

        # =========== Phase D: FFN ===========
        with ExitStack() as sd:
            pd = sd.enter_context(tc.tile_pool(name="pd", bufs=3))
            psG = sd.enter_context(tc.tile_pool(name="psG", bufs=2, space="PSUM"))
            psF = sd.enter_context(tc.tile_pool(name="psF", bufs=4, space="PSUM"))

            ps_f = [psF.tile([P, D], F32, name=f"ps_f{tq}", tag="ps_f")
                    for tq in range(TT_Q)]
            for ffc in range(FFC):
                w1t = pd.tile([P, KC_D, P], F32R, name="w1t", tag="w1t")
                nc.sync.dma_start(w1t[:], w1.ap()[:, :, ffc * P:(ffc + 1) * P])
                ps_g = psG.tile([P, QT], F32, name="ps_g", tag="ps_g")
                for kc in range(KC_D):
                    nc.tensor.matmul(
                        ps_g[:], w1t[:, kc, :], hn2T[:, kc, :],
                        start=(kc == 0), stop=(kc == KC_D - 1))
                gt = pd.tile([P, QT], F32R, name="gt", tag="gt")
                nc.scalar.activation(
                    gt[:], ps_g[:], AF.Gelu_apprx_tanh,
                    bias=b1_pc[:, ffc:ffc + 1])
                w2t = pd.tile([P, D], F32R, name="w2t", tag="w2t")
                nc.sync.dma_start(w2t[:], w2.ap()[:, ffc, :])
                for tq in range(TT_Q):
                    nc.tensor.matmul(
                        ps_f[tq][:], gt[:, tq * P:(tq + 1) * P], w2t[:],
                        start=(ffc == 0), stop=(ffc == FFC - 1))
            for tq in range(TT_Q):
                yt = pd.tile([P, D], F32, name="yt", tag="yt")
                nc.any.tensor_tensor(yt[:], ps_f[tq][:], b2_bc[:], OP.add)
                nc.any.tensor_tensor(yt[:], yt[:], h1[:, tq, :], OP.add)
                nc.sync.dma_start(y.ap()[tq * P:(tq + 1) * P, :], yt[:])


_NC_CACHE = {}


def _get_nc(loop_n=None):
    if loop_n not in _NC_CACHE:
        _NC_CACHE[loop_n] = _build_nc(loop_n)
    return _NC_CACHE[loop_n]


def _host_prep(x, pos, Wqkv, bqkv, Wo, bo, ln1_w, ln1_b, ln2_w, ln2_b,
               W1, b1, W2, b2):
    """Fold LN affines into weights, build RoPE tables, shape weights."""
    xf = np.asarray(x, np.float32).reshape(N, D)
    pf = np.asarray(pos, np.float32).reshape(N, 2)

    Wqkv = np.asarray(Wqkv, np.float32)
    W1 = np.asarray(W1, np.float32)
    wqkv_f = np.asarray(ln1_w, np.float64)[:, None] * Wqkv.astype(np.float64)
    bqkv_f = np.asarray(ln1_b, np.float64) @ Wqkv.astype(np.float64) + bqkv
    w1_f = np.asarray(ln2_w, np.float64)[:, None] * W1.astype(np.float64)
    b1_f = np.asarray(ln2_b, np.float64) @ W1.astype(np.float64) + b1

    def shape_w(w, kin):
        w = np.asarray(w, np.float32)
        ko = kin // P
        return _round_fp32r(
            np.ascontiguousarray(w.reshape(ko, P, -1).transpose(1, 0, 2)))

    wqkv_r = shape_w(wqkv_f, D)
    wo_r = shape_w(Wo, D)
    w1_r = shape_w(w1_f, D)
    w2_r = shape_w(W2, FF)

    # RoPE tables (sign-folded):  out = t*C + shuf16(t)*S
    hd2 = DH // 2
    inv_freq = (1.0 / (10000.0 ** (np.arange(0, hd2, 2, dtype=np.float64) / hd2)))
    f0 = pf[:, 0:1].astype(np.float64) * inv_freq[None, :]   # (N, 16)
    f1 = pf[:, 1:2].astype(np.float64) * inv_freq[None, :]
    C = np.concatenate(
        [np.cos(f0), np.cos(f0), np.cos(f1), np.cos(f1)], axis=1)
    S = np.concatenate(
        [-np.sin(f0), np.sin(f0), -np.sin(f1), np.sin(f1)], axis=1)
    return (xf, C.astype(np.float32), S.astype(np.float32),
            wqkv_r, np.asarray(bqkv_f, np.float32),
            wo_r, np.asarray(bo, np.float32),
            w1_r, np.asarray(b1_f, np.float32),
            w2_r, np.asarray(b2, np.float32))


def _in_maps(inputs):
    (xf, C, S, wqkv_r, bqkv_f, wo_r, bo_f, w1_r, b1_f, w2_r, b2_f) = _host_prep(
        inputs["x"], inputs["pos"], inputs["Wqkv"], inputs["bqkv"],
        inputs["Wo"], inputs["bo"], inputs["ln1_w"], inputs["ln1_b"],
        inputs["ln2_w"], inputs["ln2_b"], inputs["W1"], inputs["b1"],
        inputs["W2"], inputs["b2"])
    shared = dict(wqkv=wqkv_r, bqkv=bqkv_f, wo=wo_r, bo=bo_f,
                  w1=w1_r, b1=b1_f, w2=w2_r, b2=b2_f)
    maps = []
    for c in range(NCORES):
        seg, half = c // 2, c % 2
        own = np.arange(seg * SEG + half * QT, seg * SEG + half * QT + QT)
        other = np.arange(seg * SEG + (1 - half) * QT,
                          seg * SEG + (1 - half) * QT + QT)
        perm = np.concatenate([own, other])
        maps.append(dict(shared,
                         x_kv=np.ascontiguousarray(xf[perm]),
                         ck=np.ascontiguousarray(C[perm]),
                         sk=np.ascontiguousarray(S[perm])))
    return maps


def kernel(**inputs) -> np.ndarray:
    nc = _get_nc()
    maps = _in_maps(inputs)
    res = run_bass_kernel_spmd(nc, maps, list(range(NCORES)))
    out = np.empty((N, D), np.float32)
    for c in range(NCORES):
        seg, half = c // 2, c % 2
        q0 = seg * SEG + half * QT
        out[q0:q0 + QT] = res.results[c]["y"]
    return out.reshape(8, 512, D)
